# revision 33
# baseline (speedup 1.0000x reference)
"""Multi-head self-attention Bass kernel for TRN2, batch-parallel over 8 cores.

v2: natural-O AV orientation + fused normalize/transpose + pipelined exps.

Per-core problem (batch element b): x [T=1024, D=1024], 16 heads, d_k=64.
Dataflow (trailing T in a name = transposed layout [feature, token]):
  xT   [D, T]       host-pre-transposed input, bf16
  vg   [T, 16*65]   V natural, 65 cols/head: 64 V (strided DVE copy from
                    512-wide psum chunks) + ones col (one global memset)
  qk   [2D, T]      Q^T,K^T tiles, bf16 (written via DVE bias-add from psum)
  ST_h [T_k, T_q]   = K_h Q_h^T  (lhsT=KT_h chunk, rhs=QT_h, K=64)
  ET_h = exp(ST_h/8) bf16, one ACT op per [128, 1024] psum tile
  O_nat[q, 65]      per (pair, q-tile): psum accumulation, lhsT=ET chunk
                    (K=k-tokens), rhs=vg head cols; col 64 = row sums
  o_nat             normalized during DVE copy with per-partition 1/sums
  psT  [128, T]     O^T via identity-matmul transpose of o_nat
  ot copy           adds V bias (per-partition in O^T layout) during copy
  y    [T, D]       = lhsT=ot tile (bf16), rhs=W_o bf16 (+bias via K=1 mm)

Pipelining (window W runs): av(W) q-groups + S(W+1) tk-groups + qkt(W+2)
pieces, interleaved so PE never waits on the ACT exp drain or psum pool
rotation. The V phase is woven with pair-0 S tiles in the prologue.
"""
import numpy as np
import concourse.bacc as bacc
import concourse.mybir as mybir
from concourse.tile import TileContext
from concourse.bass import ts

F32 = mybir.dt.float32
BF16 = mybir.dt.bfloat16
AF = mybir.ActivationFunctionType

T = 1024       # tokens per core (one batch element)
D = 1024       # d_model
H = 16         # heads
DK = 64        # head dim
SCALE = 1.0 / 8.0
NT = T // 128  # 8 token tiles
ND = D // 128  # 8 d tiles
NC_T = T // 512  # 2 free-dim chunks of tokens
HWID = DK + 1      # 65: head V cols + ones col
VW = H * HWID      # 1040, augmented V width
NP = H // 2        # 8 head pairs
ET_BUFS = 5


def build_nc(repeat=1):
    nc = bacc.Bacc(None, target_bir_lowering=False, debug=False)

    xT = nc.dram_tensor("xT", [D, T], BF16, kind="ExternalInput")
    wqk = nc.dram_tensor("wqk", [2 * ND, 128, D], BF16, kind="ExternalInput")
    bqkc = nc.dram_tensor("bqkc", [128, 2 * ND], F32, kind="ExternalInput")
    wv = nc.dram_tensor("wv", [D, D], BF16, kind="ExternalInput")
    bvp = nc.dram_tensor("bvp", [128, NP], F32, kind="ExternalInput")
    wo = nc.dram_tensor("wo", [D, D], BF16, kind="ExternalInput")
    bo = nc.dram_tensor("bo", [1, D], BF16, kind="ExternalInput")
    onesd = nc.dram_tensor("onesd", [1, 128], BF16, kind="ExternalInput")
    identd = nc.dram_tensor("identd", [128, 128], BF16, kind="ExternalInput")
    y = nc.dram_tensor("y", [T, D], F32, kind="ExternalOutput")

    with TileContext(nc) as tc:
      for _rep in range(repeat):
        with (
            tc.tile_pool(name="res", bufs=1) as res,
            tc.tile_pool(name="psW", bufs=3, space="PSUM") as psW,
            tc.tile_pool(name="psO", bufs=2, space="PSUM") as psO,
        ):
            ones_t = res.tile([1, 128], BF16)
            ident_t = res.tile([128, 128], BF16)
            ot = res.tile([128, ND, T], BF16)

            with tc.tile_pool(name="qkvres", bufs=1) as qkvres:
                qk = qkvres.tile([128, 2 * ND, T], BF16)
                vg = qkvres.tile([128, NT, VW], BF16)
                vg_h = vg.rearrange("p t (h w) -> p t h w", w=HWID)

                with (
                    tc.tile_pool(name="xtp", bufs=1) as xtp,
                    tc.tile_pool(name="p2", bufs=1) as p2,
                    tc.tile_pool(name="wqkp", bufs=3) as wqkp,
                    tc.tile_pool(name="etp", bufs=ET_BUFS) as etp,
                    tc.tile_pool(name="onp", bufs=2) as onp,
                    tc.tile_pool(name="invp", bufs=4) as invp,
                ):
                    xt = xtp.tile([128, ND, T], BF16)
                    xT_r = xT.rearrange("(dt p) t -> p dt t", p=128)
                    # xt on Pool+ACT queues; SP stays free for the first wc
                    for d in range(ND):
                        [nc.gpsimd, nc.scalar][d % 2].dma_start(
                            xt[:, d, :], xT_r[:, d, :])
                    # ones columns of vg, written once (V copies skip them)
                    nc.vector.memset(vg_h[:, :, :, DK:DK + 1], 1.0)

                    wqk_r = wqk.rearrange("c p (dt f) -> c p dt f", f=128)
                    wo_r = wo.rearrange("(dt p) c -> p dt c", p=128)

                    def qkt_pieces(c, split_dma=False):
                        """Q^T/K^T c-tile as 4 interleavable pieces."""
                        wc = wqkp.tile([128, ND, 128], BF16, tag="wqk")
                        if split_dma:  # first tiles: halve DMA latency
                            nc.sync.dma_start(wc[:, 0:2, :],
                                              wqk_r[c][:, 0:2, :])
                            nc.sync.dma_start(wc[:, 2:ND, :],
                                              wqk_r[c][:, 2:ND, :])
                        else:
                            nc.sync.dma_start(wc[:], wqk_r[c])
                        state = {}

                        def quarter(i):
                            if i == 0:
                                state["pp"] = psW.tile(
                                    [128, 1024], F32, tag="wide",
                                    name=f"ppqk_{c}")
                            pp = state["pp"]
                            for d in range(i * 2, i * 2 + 2):
                                for tq in range(NC_T):
                                    nc.tensor.matmul(
                                        pp[:, ts(tq, 512)], wc[:, d, :],
                                        xt[:, d, ts(tq, 512)],
                                        start=(d == 0), stop=(d == ND - 1))
                            if i == 3:
                                nc.vector.tensor_scalar_add(
                                    qk[:, c, :], pp[:], bqk_t[:, c:c + 1])
                        return [lambda i=i: quarter(i) for i in range(4)]

                    def s_tile(hp, tk, et_pair):
                        """S matmuls + exp for head pair hp, token tile tk."""
                        for sub in range(2):
                            b0 = sub * 64
                            pss = psW.tile([128, 1024], F32, tag="wide",
                                           name=f"pss_{hp}_{tk}_{sub}")
                            for tq in range(NC_T):
                                nc.tensor.matmul(
                                    pss[:, ts(tq, 512)],
                                    qk[b0:b0 + DK, ND + hp, ts(tk, 128)],
                                    qk[b0:b0 + DK, hp, ts(tq, 512)],
                                    start=True, stop=True,
                                    tile_position=(b0, 0))
                            nc.scalar.activation(
                                et_pair[sub][:, tk, :], pss[:], AF.Exp,
                                scale=SCALE)

                    et_bufs = [
                        [etp.tile([128, NT, T], BF16, tag="et",
                                  name=f"et_{hp}_{s}") for s in range(2)]
                        for hp in range(NP)
                    ]

                    # ---- prologue: V phase woven with pair-0/1 S + qkt ----
                    with tc.tile_pool(name="wvp", bufs=1) as wvp:
                        wvt = wvp.tile([128, ND, D], BF16)
                        wv_r = wv.rearrange("(dt p) c -> p dt c", p=128)
                        pieces01 = (qkt_pieces(0, split_dma=True) +
                                    qkt_pieces(ND, split_dma=True))
                        for d in range(ND):
                            [nc.gpsimd, nc.scalar][d % 2].dma_start(
                                wvt[:, d, :], wv_r[:, d, :])
                        bqk_t = p2.tile([128, 2 * ND], F32)
                        nc.sync.dma_start(bqk_t[:], bqkc[:])
                        bvp_t = p2.tile([128, NP], F32)
                        nc.sync.dma_start(bvp_t[:], bvp[:])
                        bo_t = qkvres.tile([1, D], BF16)
                        nc.sync.dma_start(bo_t[:], bo[:])
                        nc.sync.dma_start(ones_t[:], onesd[:])
                        nc.sync.dma_start(ident_t[:], identd[:])
                        for f in pieces01:
                            f()
                        extra = qkt_pieces(1) + qkt_pieces(ND + 1)
                        for t in range(NT):
                            if t > 0:  # S shifted one slot: V(0) covers the
                                s_tile(0, t - 1, et_bufs[0])  # qk bias-add
                            # V projection for token tile t: two 512 chunks
                            for ch in range(2):
                                pp = psW.tile([128, 1024], F32, tag="wide")
                                for d in range(ND):
                                    nc.tensor.matmul(
                                        pp[:, :512], xt[:, d, ts(t, 128)],
                                        wvt[:, d, ts(ch, 512)],
                                        start=(d == 0), stop=(d == ND - 1))
                                pp_h = pp.rearrange("p (h w) -> p h w", w=DK)
                                nc.vector.tensor_copy(
                                    vg_h[:, t, ch * 8:ch * 8 + 8, 0:DK],
                                    pp_h[:, 0:8, :])
                            if extra[t] is not None:
                                extra[t]()
                        s_tile(0, NT - 1, et_bufs[0])

                    # broadcast bias tile: y bias via DVE add, not mms
                    bo_bc = qkvres.tile([128, D], BF16)
                    pbps = psW.tile([128, 1024], F32, tag="wide",
                                    name="pb_bias")
                    for oc in range(NC_T):
                        nc.tensor.matmul(pbps[:, ts(oc, 512)], ones_t[:],
                                         bo_t[:, ts(oc, 512)],
                                         start=True, stop=True)
                    nc.vector.tensor_copy(bo_bc[:], pbps[:])

                    # ---- steady windows W: av(W) | S(W+1) | qkt(W+2) ----
                    wo_t = qkvres.tile([128, ND, D], BF16)
                    py_tiles = {}

                    def y_part(t, hi):
                        if t not in py_tiles:
                            py_tiles[t] = [psW.tile([128, 1024], F32,
                                                    tag="wide",
                                                    name=f"py_{t}"), 0]
                        ent = py_tiles[t]
                        for oc in range(NC_T):
                            for d in range(ent[1], hi):
                                nc.tensor.matmul(
                                    ent[0][:, ts(oc, 512)],
                                    ot[:, d, ts(t, 128)],
                                    wo_t[:, d, ts(oc, 512)],
                                    start=(d == 0), stop=False)
                        ent[1] = hi

                    def av_q(W, q, on_t):
                        et_pair = et_bufs[W]
                        po = psO.tile([128, 2 * HWID], F32, tag="po",
                                      name=f"po_{W}_{q}")
                        for sub in range(2):
                            h = 2 * W + sub
                            for kt in range(NT):
                                nc.tensor.matmul(
                                    po[:, sub * HWID:(sub + 1) * HWID],
                                    et_pair[sub][:, kt, ts(q, 128)],
                                    vg[:, kt, h * HWID:(h + 1) * HWID],
                                    start=(kt == 0), stop=(kt == NT - 1))
                        iv = invp.tile([128, 2], F32, tag="inv",
                                       name=f"inv_{W}_{q}")
                        for sub in range(2):
                            nc.vector.reciprocal(
                                iv[:, sub:sub + 1],
                                po[:, sub * HWID + DK:sub * HWID + DK + 1])
                        for sub in range(2):
                            nc.vector.tensor_scalar_mul(
                                on_t[:, q, sub * DK:(sub + 1) * DK],
                                po[:, sub * HWID:sub * HWID + DK],
                                iv[:, sub:sub + 1])

                    def trans_half(W, on_t, st, half):
                        if half == 0:
                            st["psT"] = psW.tile([128, 1024], F32,
                                                 tag="wide", name=f"psT_{W}")
                        psT = st["psT"]
                        for q in range(half * 4, half * 4 + 4):
                            nc.tensor.matmul(
                                psT[:, ts(q, 128)], on_t[:, q, :],
                                ident_t[:], start=True, stop=True)
                        if half == 1:
                            for hh in range(2):  # halves: y's d-mm for the
                                # first token chunks unblocks sooner
                                nc.vector.tensor_scalar_add(
                                    ot[:, W, ts(hh, 512)],
                                    psT[:, ts(hh, 512)], bvp_t[:, W:W + 1])

                    for W in range(NP):
                        on_t = onp.tile([128, NT, 128], BF16, tag="onat",
                                        name=f"onat_{W}")
                        s_p = ([(lambda tk=tk:
                                 s_tile(W + 1, tk, et_bufs[W + 1]))
                                for tk in range(NT)] if W + 1 < NP
                               else [None] * NT)
                        q_p = (qkt_pieces(W + 2) + qkt_pieces(ND + W + 2)
                               if W + 2 < NP else [None] * 8)
                        av = [lambda q=q, W=W, o=on_t: av_q(W, q, o)
                              for q in range(NT)]
                        st = {}
                        trA = (lambda W=W, o=on_t, s=st:
                               trans_half(W, o, s, 0))
                        trB = (lambda W=W, o=on_t, s=st:
                               trans_half(W, o, s, 1))
                        if W < NP - 2:
                            order = [q_p[0], s_p[0], q_p[1], av[0],
                                     s_p[1], q_p[2], av[1], s_p[2],
                                     q_p[3], av[2], s_p[3], q_p[4],
                                     av[3], s_p[4], q_p[5], av[4],
                                     s_p[5], q_p[6], av[5], s_p[6],
                                     q_p[7], av[6], s_p[7], av[7],
                                     trA, trB]
                        elif W == NP - 2:
                            # no more qkt work: fill with partial y groups
                            # (d0..d5 need only pairs 0-5); tks 0-1 of S(7)
                            # were already issued at window 5's end
                            order = [s_p[2], av[0], s_p[3], av[1],
                                     s_p[4], av[2], av[3],
                                     lambda: y_part(0, 3),
                                     s_p[5], av[4],
                                     lambda: y_part(0, 6),
                                     s_p[6], av[5],
                                     s_p[7], av[6], av[7], trA, trB]
                        else:
                            # last pair: fill exp-wait stalls with partial
                            # y-projection groups (d<=6 need pairs 0-6 only)
                            order = [av[0], av[1], av[2],
                                     lambda: y_part(1, 3),
                                     av[3], lambda: y_part(1, 6),
                                     av[4], lambda: y_part(0, 7),
                                     av[5], av[6],
                                     lambda: y_part(1, 7),
                                     av[7], trA, trB]
                        for f in order:
                            if f is not None:
                                f()
                        if W == NP - 3:  # prefetch W_o during tail pairs
                            for d in range(ND):
                                nc.gpsimd.dma_start(wo_t[:, d, :],
                                                    wo_r[:, d, :])
                            # feed ACT across the boundary into window 6
                            s_tile(NP - 1, 0, et_bufs[NP - 1])
                            s_tile(NP - 1, 1, et_bufs[NP - 1])

                # ---- output projection (all bf16) ----
                with tc.tile_pool(name="yp", bufs=4) as yp:
                    # t=2 first, from the freed psO banks: its d0..d6 mms
                    # cover the trans(7)->ot copy latency while the wide
                    # pool is still pinned by the window-7 partials
                    p2a = psO.tile([128, 512], F32, tag="po", name="p2a")
                    p2b = psO.tile([128, 512], F32, tag="po", name="p2b")
                    for oc, pyx in ((0, p2a), (1, p2b)):
                        for d in range(ND - 1):
                            nc.tensor.matmul(
                                pyx[:], ot[:, d, ts(2, 128)],
                                wo_t[:, d, ts(oc, 512)],
                                start=(d == 0), stop=False)
                    for t in [2, 0, 1, 3, 4, 5, 6, 7]:
                        if t == 2:
                            grp = [(0, p2a, ND - 1), (1, p2b, ND - 1)]
                        elif t in py_tiles:
                            py, d0 = py_tiles[t]
                            grp = [(oc, py[:, ts(oc, 512)], d0)
                                   for oc in range(NC_T)]
                        else:
                            py = psW.tile([128, 1024], F32, tag="wide",
                                          name=f"pyf_{t}")
                            grp = [(oc, py[:, ts(oc, 512)], 0)
                                   for oc in range(NC_T)]
                        for oc, pyx, d0 in grp:
                            for d in range(d0, ND):
                                nc.tensor.matmul(
                                    pyx[:], ot[:, d, ts(t, 128)],
                                    wo_t[:, d, ts(oc, 512)],
                                    start=(d == 0), stop=(d == ND - 1))
                            yt = yp.tile([128, 512], F32, tag="yt")
                            nc.vector.tensor_add(yt[:], pyx[:],
                                                 bo_bc[:, ts(oc, 512)])
                            nc.sync.dma_start(
                                y[ts(t, 128), ts(oc, 512)], yt[:])

    nc.finalize()
    return nc


def prep_in_maps(x, W_qkv, b_qkv, W_o, b_o):
    """Host-side sharding: batch-parallel, one batch element per core."""
    import ml_dtypes
    bf16 = ml_dtypes.bfloat16
    B = x.shape[0]
    # wqk relayout: [2ND c-tiles, 128 partitions(cols), D contiguous]
    W_qk = np.asarray(W_qkv[:, :2 * D], np.float32)
    wqk_t = np.transpose(
        W_qk.reshape(ND, 128, 2 * ND, 128), (2, 1, 0, 3)
    ).reshape(2 * ND, 128, D)  # [c, row-in-dtile(p), d-tile*128+col]
    b_qkc = np.ascontiguousarray(
        np.asarray(b_qkv[:2 * D], np.float32).reshape(2 * ND, 128).T)
    W_vo = np.ascontiguousarray(W_qkv[:, 2 * D:])    # [D, D] V weights
    b_vo = np.asarray(b_qkv[2 * D:], np.float32)
    bvp_a = np.ascontiguousarray(b_vo.reshape(NP, 128).T)
    ones = np.ones((1, 128), bf16)
    ident = np.eye(128, dtype=np.float32).astype(bf16)
    in_maps = []
    for b in range(B):
        in_maps.append({
            "xT": np.ascontiguousarray(x[b].T).astype(bf16),
            "wqk": np.ascontiguousarray(wqk_t).astype(bf16),
            "bqkc": b_qkc, "bvp": bvp_a,
            "wv": W_vo.astype(bf16),
            "wo": np.ascontiguousarray(W_o).astype(bf16),
            "bo": np.ascontiguousarray(b_o).reshape(1, -1).astype(bf16),
            "onesd": ones, "identd": ident,
        })
    return in_maps


# ---------------------------------------------------------------------------
# Self-contained SPMD runner (axon PJRT path) and the graded entry point.
# ---------------------------------------------------------------------------
import jax as _jax


_CACHE = {}


def _make_runner(nc, n_cores=8):
    from jax.sharding import Mesh, PartitionSpec
    from jax.experimental.shard_map import shard_map
    from concourse import bass2jax

    bass2jax.install_neuronx_cc_hook()
    partition_name = nc.partition_id_tensor.name if nc.partition_id_tensor else None
    in_names, out_names, out_avals, zero_outs = [], [], [], []
    for alloc in nc.m.functions[0].allocations:
        if not isinstance(alloc, mybir.MemoryLocationSet):
            continue
        name = alloc.memorylocations[0].name
        if alloc.kind == "ExternalInput":
            if name != partition_name:
                in_names.append(name)
        elif alloc.kind == "ExternalOutput":
            shape = tuple(alloc.tensor_shape)
            dtype = mybir.dt.np(alloc.dtype)
            out_names.append(name)
            out_avals.append(_jax.core.ShapedArray(shape, dtype))
            zero_outs.append(np.zeros(shape, dtype))
    n_params = len(in_names)
    all_in_names = list(in_names) + list(out_names)
    if partition_name is not None:
        all_in_names.append(partition_name)

    def _body(*args):
        operands = list(args)
        if partition_name is not None:
            operands.append(bass2jax.partition_id_tensor())
        return tuple(bass2jax._bass_exec_p.bind(
            *operands,
            out_avals=tuple(out_avals),
            in_names=tuple(all_in_names),
            out_names=tuple(out_names),
            lowering_input_output_aliases=(),
            sim_require_finite=True,
            sim_require_nnan=True,
            nc=nc,
        ))

    devices = _jax.devices()[:n_cores]
    mesh = Mesh(np.asarray(devices), ("core",))
    nin = n_params + len(out_names)
    sharded = _jax.jit(
        shard_map(_body, mesh=mesh,
                  in_specs=(PartitionSpec("core"),) * nin,
                  out_specs=(PartitionSpec("core"),) * len(out_names),
                  check_rep=False),
        keep_unused=True,
    )

    def run(in_maps):
        concat_in = [
            np.concatenate([np.asarray(m[name]) for m in in_maps], axis=0)
            for name in in_names
        ]
        concat_zeros = [
            np.zeros((n_cores * z.shape[0], *z.shape[1:]), z.dtype)
            for z in zero_outs
        ]
        out_arrs = [np.asarray(o) for o in sharded(*concat_in, *concat_zeros)]
        return [
            {name: out_arrs[i].reshape(n_cores, *out_avals[i].shape)[c]
             for i, name in enumerate(out_names)}
            for c in range(n_cores)
        ]

    return run


def kernel(x, W_qkv, b_qkv, W_o, b_o):
    """Full-input entry point: shards batch across the 8 NeuronCores,
    runs the Bass MHA kernel SPMD, gathers the full output."""
    x = np.ascontiguousarray(np.asarray(x, np.float32))
    W_qkv = np.asarray(W_qkv, np.float32)
    b_qkv = np.asarray(b_qkv, np.float32)
    W_o = np.asarray(W_o, np.float32)
    b_o = np.asarray(b_o, np.float32)
    B = x.shape[0]
    assert x.shape == (8, T, D), f"unexpected x shape {x.shape}"

    if "run" not in _CACHE:
        nc = build_nc()
        _CACHE["run"] = _make_runner(nc, n_cores=8)
    run = _CACHE["run"]

    in_maps = prep_in_maps(x, W_qkv, b_qkv, W_o, b_o)
    res = run(in_maps)
    out = np.stack([res[b]["y"] for b in range(B)]).astype(np.float32)
    return out



# revision 34
# speedup vs baseline: 1.0007x; 1.0007x over previous
"""Multi-head self-attention Bass kernel for TRN2, batch-parallel over 8 cores.

v2: natural-O AV orientation + fused normalize/transpose + pipelined exps.

Per-core problem (batch element b): x [T=1024, D=1024], 16 heads, d_k=64.
Dataflow (trailing T in a name = transposed layout [feature, token]):
  xT   [D, T]       host-pre-transposed input, bf16
  vg   [T, 16*65]   V natural, 65 cols/head: 64 V (strided DVE copy from
                    512-wide psum chunks) + ones col (one global memset)
  qk   [2D, T]      Q^T,K^T tiles, bf16 (written via DVE bias-add from psum)
  ST_h [T_k, T_q]   = K_h Q_h^T  (lhsT=KT_h chunk, rhs=QT_h, K=64)
  ET_h = exp(ST_h/8) bf16, one ACT op per [128, 1024] psum tile
  O_nat[q, 65]      per (pair, q-tile): psum accumulation, lhsT=ET chunk
                    (K=k-tokens), rhs=vg head cols; col 64 = row sums
  o_nat             normalized during DVE copy with per-partition 1/sums
  psT  [128, T]     O^T via identity-matmul transpose of o_nat
  ot copy           adds V bias (per-partition in O^T layout) during copy
  y    [T, D]       = lhsT=ot tile (bf16), rhs=W_o bf16 (+bias via K=1 mm)

Pipelining (window W runs): av(W) q-groups + S(W+1) tk-groups + qkt(W+2)
pieces, interleaved so PE never waits on the ACT exp drain or psum pool
rotation. The V phase is woven with pair-0 S tiles in the prologue.
"""
import numpy as np
import concourse.bacc as bacc
import concourse.mybir as mybir
from concourse.tile import TileContext
from concourse.bass import ts

F32 = mybir.dt.float32
BF16 = mybir.dt.bfloat16
AF = mybir.ActivationFunctionType

T = 1024       # tokens per core (one batch element)
D = 1024       # d_model
H = 16         # heads
DK = 64        # head dim
SCALE = 1.0 / 8.0
NT = T // 128  # 8 token tiles
ND = D // 128  # 8 d tiles
NC_T = T // 512  # 2 free-dim chunks of tokens
HWID = DK + 1      # 65: head V cols + ones col
VW = H * HWID      # 1040, augmented V width
NP = H // 2        # 8 head pairs
ET_BUFS = 5


def build_nc(repeat=1):
    nc = bacc.Bacc(None, target_bir_lowering=False, debug=False)

    xT = nc.dram_tensor("xT", [D, T], BF16, kind="ExternalInput")
    wqk = nc.dram_tensor("wqk", [2 * ND, 128, D], BF16, kind="ExternalInput")
    bqkc = nc.dram_tensor("bqkc", [128, 2 * ND], F32, kind="ExternalInput")
    wv = nc.dram_tensor("wv", [D, D], BF16, kind="ExternalInput")
    bvp = nc.dram_tensor("bvp", [128, NP], F32, kind="ExternalInput")
    wo = nc.dram_tensor("wo", [D, D], BF16, kind="ExternalInput")
    bo = nc.dram_tensor("bo", [1, D], BF16, kind="ExternalInput")
    onesd = nc.dram_tensor("onesd", [1, 128], BF16, kind="ExternalInput")
    identd = nc.dram_tensor("identd", [128, 128], BF16, kind="ExternalInput")
    y = nc.dram_tensor("y", [T, D], F32, kind="ExternalOutput")

    with TileContext(nc) as tc:
      for _rep in range(repeat):
        with (
            tc.tile_pool(name="res", bufs=1) as res,
            tc.tile_pool(name="psW", bufs=3, space="PSUM") as psW,
            tc.tile_pool(name="psO", bufs=2, space="PSUM") as psO,
        ):
            ones_t = res.tile([1, 128], BF16)
            ident_t = res.tile([128, 128], BF16)
            ot = res.tile([128, ND, T], BF16)

            with tc.tile_pool(name="qkvres", bufs=1) as qkvres:
                qk = qkvres.tile([128, 2 * ND, T], BF16)
                vg = qkvres.tile([128, NT, VW], BF16)
                vg_h = vg.rearrange("p t (h w) -> p t h w", w=HWID)

                with (
                    tc.tile_pool(name="xtp", bufs=1) as xtp,
                    tc.tile_pool(name="p2", bufs=1) as p2,
                    tc.tile_pool(name="wqkp", bufs=3) as wqkp,
                    tc.tile_pool(name="etp", bufs=ET_BUFS) as etp,
                    tc.tile_pool(name="onp", bufs=2) as onp,
                    tc.tile_pool(name="invp", bufs=4) as invp,
                ):
                    xt = xtp.tile([128, ND, T], BF16)
                    xT_r = xT.rearrange("(dt p) t -> p dt t", p=128)
                    # xt on Pool+ACT queues; SP stays free for the first wc
                    for d in range(ND):
                        [nc.gpsimd, nc.scalar][d % 2].dma_start(
                            xt[:, d, :], xT_r[:, d, :])
                    # ones columns of vg, written once (V copies skip them)
                    nc.vector.memset(vg_h[:, :, :, DK:DK + 1], 1.0)

                    wqk_r = wqk.rearrange("c p (dt f) -> c p dt f", f=128)
                    wo_r = wo.rearrange("(dt p) c -> p dt c", p=128)

                    def qkt_pieces(c, split_dma=False):
                        """Q^T/K^T c-tile as 4 interleavable pieces."""
                        wc = wqkp.tile([128, ND, 128], BF16, tag="wqk")
                        if split_dma:  # first tiles: halve DMA latency
                            nc.sync.dma_start(wc[:, 0:2, :],
                                              wqk_r[c][:, 0:2, :])
                            nc.sync.dma_start(wc[:, 2:ND, :],
                                              wqk_r[c][:, 2:ND, :])
                        else:
                            nc.sync.dma_start(wc[:], wqk_r[c])
                        state = {}

                        def quarter(i):
                            if i == 0:
                                state["pp"] = psW.tile(
                                    [128, 1024], F32, tag="wide",
                                    name=f"ppqk_{c}")
                            pp = state["pp"]
                            for d in range(i * 2, i * 2 + 2):
                                for tq in range(NC_T):
                                    nc.tensor.matmul(
                                        pp[:, ts(tq, 512)], wc[:, d, :],
                                        xt[:, d, ts(tq, 512)],
                                        start=(d == 0), stop=(d == ND - 1))
                            if i == 3:
                                nc.vector.tensor_scalar_add(
                                    qk[:, c, :], pp[:], bqk_t[:, c:c + 1])
                        return [lambda i=i: quarter(i) for i in range(4)]

                    def s_tile(hp, tk, et_pair):
                        """S matmuls + exp for head pair hp, token tile tk."""
                        for sub in range(2):
                            b0 = sub * 64
                            pss = psW.tile([128, 1024], F32, tag="wide",
                                           name=f"pss_{hp}_{tk}_{sub}")
                            for tq in range(NC_T):
                                nc.tensor.matmul(
                                    pss[:, ts(tq, 512)],
                                    qk[b0:b0 + DK, ND + hp, ts(tk, 128)],
                                    qk[b0:b0 + DK, hp, ts(tq, 512)],
                                    start=True, stop=True,
                                    tile_position=(b0, 0))
                            nc.scalar.activation(
                                et_pair[sub][:, tk, :], pss[:], AF.Exp,
                                scale=SCALE)

                    et_bufs = [
                        [etp.tile([128, NT, T], BF16, tag="et",
                                  name=f"et_{hp}_{s}") for s in range(2)]
                        for hp in range(NP)
                    ]

                    # ---- prologue: V phase woven with pair-0/1 S + qkt ----
                    with tc.tile_pool(name="wvp", bufs=1) as wvp:
                        wvt = wvp.tile([128, ND, D], BF16)
                        wv_r = wv.rearrange("(dt p) c -> p dt c", p=128)
                        pieces01 = (qkt_pieces(0, split_dma=True) +
                                    qkt_pieces(ND, split_dma=True))
                        for d in range(ND):
                            [nc.gpsimd, nc.scalar][d % 2].dma_start(
                                wvt[:, d, :], wv_r[:, d, :])
                        bqk_t = p2.tile([128, 2 * ND], F32)
                        nc.sync.dma_start(bqk_t[:], bqkc[:])
                        bvp_t = p2.tile([128, NP], F32)
                        nc.sync.dma_start(bvp_t[:], bvp[:])
                        bo_t = qkvres.tile([1, D], BF16)
                        nc.sync.dma_start(bo_t[:], bo[:])
                        nc.sync.dma_start(ones_t[:], onesd[:])
                        nc.sync.dma_start(ident_t[:], identd[:])
                        for f in pieces01:
                            f()
                        extra = qkt_pieces(1) + qkt_pieces(ND + 1)
                        for t in range(NT):
                            if t > 0:  # S shifted one slot: V(0) covers the
                                s_tile(0, t - 1, et_bufs[0])  # qk bias-add
                            # V projection for token tile t: two 512 chunks
                            for ch in range(2):
                                pp = psW.tile([128, 1024], F32, tag="wide")
                                for d in range(ND):
                                    nc.tensor.matmul(
                                        pp[:, :512], xt[:, d, ts(t, 128)],
                                        wvt[:, d, ts(ch, 512)],
                                        start=(d == 0), stop=(d == ND - 1))
                                pp_h = pp.rearrange("p (h w) -> p h w", w=DK)
                                nc.vector.tensor_copy(
                                    vg_h[:, t, ch * 8:ch * 8 + 8, 0:DK],
                                    pp_h[:, 0:8, :])
                            if extra[t] is not None:
                                extra[t]()
                        s_tile(0, NT - 1, et_bufs[0])

                    # broadcast bias tile: y bias via DVE add, not mms
                    bo_bc = qkvres.tile([128, D], BF16)
                    pbps = psW.tile([128, 1024], F32, tag="wide",
                                    name="pb_bias")
                    for oc in range(NC_T):
                        nc.tensor.matmul(pbps[:, ts(oc, 512)], ones_t[:],
                                         bo_t[:, ts(oc, 512)],
                                         start=True, stop=True)
                    nc.vector.tensor_copy(bo_bc[:], pbps[:])

                    # ---- steady windows W: av(W) | S(W+1) | qkt(W+2) ----
                    wo_t = qkvres.tile([128, ND, D], BF16)
                    py_tiles = {}

                    def y_part(t, hi):
                        if t not in py_tiles:
                            py_tiles[t] = [psW.tile([128, 1024], F32,
                                                    tag="wide",
                                                    name=f"py_{t}"), 0]
                        ent = py_tiles[t]
                        for oc in range(NC_T):
                            for d in range(ent[1], hi):
                                nc.tensor.matmul(
                                    ent[0][:, ts(oc, 512)],
                                    ot[:, d, ts(t, 128)],
                                    wo_t[:, d, ts(oc, 512)],
                                    start=(d == 0), stop=False)
                        ent[1] = hi

                    def av_q(W, q, on_t):
                        et_pair = et_bufs[W]
                        po = psO.tile([128, 2 * HWID], F32, tag="po",
                                      name=f"po_{W}_{q}")
                        for sub in range(2):
                            h = 2 * W + sub
                            for kt in range(NT):
                                nc.tensor.matmul(
                                    po[:, sub * HWID:(sub + 1) * HWID],
                                    et_pair[sub][:, kt, ts(q, 128)],
                                    vg[:, kt, h * HWID:(h + 1) * HWID],
                                    start=(kt == 0), stop=(kt == NT - 1))
                        iv = invp.tile([128, 2], F32, tag="inv",
                                       name=f"inv_{W}_{q}")
                        for sub in range(2):
                            nc.vector.reciprocal(
                                iv[:, sub:sub + 1],
                                po[:, sub * HWID + DK:sub * HWID + DK + 1])
                        for sub in range(2):
                            nc.vector.tensor_scalar_mul(
                                on_t[:, q, sub * DK:(sub + 1) * DK],
                                po[:, sub * HWID:sub * HWID + DK],
                                iv[:, sub:sub + 1])

                    def trans_half(W, on_t, st, half):
                        if half == 0:
                            st["psT"] = psW.tile([128, 1024], F32,
                                                 tag="wide", name=f"psT_{W}")
                        psT = st["psT"]
                        for q in range(half * 4, half * 4 + 4):
                            nc.tensor.matmul(
                                psT[:, ts(q, 128)], on_t[:, q, :],
                                ident_t[:], start=True, stop=True)
                        if half == 1:
                            for hh in range(2):  # halves: y's d-mm for the
                                # first token chunks unblocks sooner
                                nc.vector.tensor_scalar_add(
                                    ot[:, W, ts(hh, 512)],
                                    psT[:, ts(hh, 512)], bvp_t[:, W:W + 1])

                    for W in range(NP):
                        on_t = onp.tile([128, NT, 128], BF16, tag="onat",
                                        name=f"onat_{W}")
                        s_p = ([(lambda tk=tk:
                                 s_tile(W + 1, tk, et_bufs[W + 1]))
                                for tk in range(NT)] if W + 1 < NP
                               else [None] * NT)
                        q_p = (qkt_pieces(W + 2) + qkt_pieces(ND + W + 2)
                               if W + 2 < NP else [None] * 8)
                        av = [lambda q=q, W=W, o=on_t: av_q(W, q, o)
                              for q in range(NT)]
                        st = {}
                        trA = (lambda W=W, o=on_t, s=st:
                               trans_half(W, o, s, 0))
                        trB = (lambda W=W, o=on_t, s=st:
                               trans_half(W, o, s, 1))
                        if W < NP - 2:
                            order = [q_p[0], s_p[0], q_p[1], av[0],
                                     s_p[1], q_p[2], av[1], s_p[2],
                                     q_p[3], av[2], s_p[3], q_p[4],
                                     av[3], s_p[4], q_p[5], av[4],
                                     s_p[5], q_p[6], av[5], s_p[6],
                                     q_p[7], av[6], s_p[7], av[7],
                                     trA, trB]
                        elif W == NP - 2:
                            # no more qkt work: fill with partial y groups
                            # (d0..d5 need only pairs 0-5); tks 0-1 of S(7)
                            # were already issued at window 5's end
                            order = [s_p[2], av[0], s_p[3], av[1],
                                     s_p[4], av[2], av[3],
                                     lambda: y_part(0, 3),
                                     s_p[5], av[4],
                                     lambda: y_part(0, 6),
                                     s_p[6], av[5],
                                     s_p[7], av[6], av[7], trA, trB]
                        else:
                            # last pair: fill exp-wait stalls with partial
                            # y-projection groups (d<=6 need pairs 0-6 only)
                            order = [av[0], av[1], av[2],
                                     lambda: y_part(1, 3),
                                     av[3], lambda: y_part(1, 6),
                                     av[4], lambda: y_part(0, 7),
                                     av[5], av[6],
                                     lambda: y_part(1, 7),
                                     av[7], trA, trB]
                        for f in order:
                            if f is not None:
                                f()
                        if W == NP - 3:  # prefetch W_o during tail pairs
                            for d in range(ND):
                                nc.gpsimd.dma_start(wo_t[:, d, :],
                                                    wo_r[:, d, :])
                            # feed ACT across the boundary into window 6
                            s_tile(NP - 1, 0, et_bufs[NP - 1])
                            s_tile(NP - 1, 1, et_bufs[NP - 1])

                # ---- output projection (all bf16) ----
                with tc.tile_pool(name="yp", bufs=4) as yp:
                    # t=2 first, from the freed psO banks: its d0..d6 mms
                    # cover the trans(7)->ot copy latency while the wide
                    # pool is still pinned by the window-7 partials
                    p2a = psO.tile([128, 512], F32, tag="po", name="p2a")
                    p2b = psO.tile([128, 512], F32, tag="po", name="p2b")
                    for oc, pyx in ((0, p2a), (1, p2b)):
                        for d in range(ND - 1):
                            nc.tensor.matmul(
                                pyx[:], ot[:, d, ts(2, 128)],
                                wo_t[:, d, ts(oc, 512)],
                                start=(d == 0), stop=False)
                    for t in [2, 0, 1, 3, 4, 5, 6, 7]:
                        if t == 2:
                            grp = [(0, p2a, ND - 1), (1, p2b, ND - 1)]
                        elif t in py_tiles:
                            py, d0 = py_tiles[t]
                            grp = [(oc, py[:, ts(oc, 512)], d0)
                                   for oc in range(NC_T)]
                        else:
                            py = psW.tile([128, 1024], F32, tag="wide",
                                          name=f"pyf_{t}")
                            grp = [(oc, py[:, ts(oc, 512)], 0)
                                   for oc in range(NC_T)]
                        for oc, pyx, d0 in grp:
                            use_act = (t % 2 == 1)  # stage odd tiles on ACT
                            for d in range(d0, ND):
                                nc.tensor.matmul(
                                    pyx[:], ot[:, d, ts(t, 128)],
                                    wo_t[:, d, ts(oc, 512)],
                                    start=(d == 0),
                                    stop=(d == ND - 1) and not use_act)
                            yt = yp.tile([128, 512], F32, tag="yt")
                            if use_act:  # bias via mm, copy on idle ACT
                                nc.tensor.matmul(
                                    pyx[:], ones_t[:], bo_t[:, ts(oc, 512)],
                                    start=False, stop=True)
                                nc.scalar.copy(yt[:], pyx[:])
                            else:
                                nc.vector.tensor_add(yt[:], pyx[:],
                                                     bo_bc[:, ts(oc, 512)])
                            nc.sync.dma_start(
                                y[ts(t, 128), ts(oc, 512)], yt[:])

    nc.finalize()
    return nc


def prep_in_maps(x, W_qkv, b_qkv, W_o, b_o):
    """Host-side sharding: batch-parallel, one batch element per core."""
    import ml_dtypes
    bf16 = ml_dtypes.bfloat16
    B = x.shape[0]
    # wqk relayout: [2ND c-tiles, 128 partitions(cols), D contiguous]
    W_qk = np.asarray(W_qkv[:, :2 * D], np.float32)
    wqk_t = np.transpose(
        W_qk.reshape(ND, 128, 2 * ND, 128), (2, 1, 0, 3)
    ).reshape(2 * ND, 128, D)  # [c, row-in-dtile(p), d-tile*128+col]
    b_qkc = np.ascontiguousarray(
        np.asarray(b_qkv[:2 * D], np.float32).reshape(2 * ND, 128).T)
    W_vo = np.ascontiguousarray(W_qkv[:, 2 * D:])    # [D, D] V weights
    b_vo = np.asarray(b_qkv[2 * D:], np.float32)
    bvp_a = np.ascontiguousarray(b_vo.reshape(NP, 128).T)
    ones = np.ones((1, 128), bf16)
    ident = np.eye(128, dtype=np.float32).astype(bf16)
    in_maps = []
    for b in range(B):
        in_maps.append({
            "xT": np.ascontiguousarray(x[b].T).astype(bf16),
            "wqk": np.ascontiguousarray(wqk_t).astype(bf16),
            "bqkc": b_qkc, "bvp": bvp_a,
            "wv": W_vo.astype(bf16),
            "wo": np.ascontiguousarray(W_o).astype(bf16),
            "bo": np.ascontiguousarray(b_o).reshape(1, -1).astype(bf16),
            "onesd": ones, "identd": ident,
        })
    return in_maps


# ---------------------------------------------------------------------------
# Self-contained SPMD runner (axon PJRT path) and the graded entry point.
# ---------------------------------------------------------------------------
import jax as _jax


_CACHE = {}


def _make_runner(nc, n_cores=8):
    from jax.sharding import Mesh, PartitionSpec
    from jax.experimental.shard_map import shard_map
    from concourse import bass2jax

    bass2jax.install_neuronx_cc_hook()
    partition_name = nc.partition_id_tensor.name if nc.partition_id_tensor else None
    in_names, out_names, out_avals, zero_outs = [], [], [], []
    for alloc in nc.m.functions[0].allocations:
        if not isinstance(alloc, mybir.MemoryLocationSet):
            continue
        name = alloc.memorylocations[0].name
        if alloc.kind == "ExternalInput":
            if name != partition_name:
                in_names.append(name)
        elif alloc.kind == "ExternalOutput":
            shape = tuple(alloc.tensor_shape)
            dtype = mybir.dt.np(alloc.dtype)
            out_names.append(name)
            out_avals.append(_jax.core.ShapedArray(shape, dtype))
            zero_outs.append(np.zeros(shape, dtype))
    n_params = len(in_names)
    all_in_names = list(in_names) + list(out_names)
    if partition_name is not None:
        all_in_names.append(partition_name)

    def _body(*args):
        operands = list(args)
        if partition_name is not None:
            operands.append(bass2jax.partition_id_tensor())
        return tuple(bass2jax._bass_exec_p.bind(
            *operands,
            out_avals=tuple(out_avals),
            in_names=tuple(all_in_names),
            out_names=tuple(out_names),
            lowering_input_output_aliases=(),
            sim_require_finite=True,
            sim_require_nnan=True,
            nc=nc,
        ))

    devices = _jax.devices()[:n_cores]
    mesh = Mesh(np.asarray(devices), ("core",))
    nin = n_params + len(out_names)
    sharded = _jax.jit(
        shard_map(_body, mesh=mesh,
                  in_specs=(PartitionSpec("core"),) * nin,
                  out_specs=(PartitionSpec("core"),) * len(out_names),
                  check_rep=False),
        keep_unused=True,
    )

    def run(in_maps):
        concat_in = [
            np.concatenate([np.asarray(m[name]) for m in in_maps], axis=0)
            for name in in_names
        ]
        concat_zeros = [
            np.zeros((n_cores * z.shape[0], *z.shape[1:]), z.dtype)
            for z in zero_outs
        ]
        out_arrs = [np.asarray(o) for o in sharded(*concat_in, *concat_zeros)]
        return [
            {name: out_arrs[i].reshape(n_cores, *out_avals[i].shape)[c]
             for i, name in enumerate(out_names)}
            for c in range(n_cores)
        ]

    return run


def kernel(x, W_qkv, b_qkv, W_o, b_o):
    """Full-input entry point: shards batch across the 8 NeuronCores,
    runs the Bass MHA kernel SPMD, gathers the full output."""
    x = np.ascontiguousarray(np.asarray(x, np.float32))
    W_qkv = np.asarray(W_qkv, np.float32)
    b_qkv = np.asarray(b_qkv, np.float32)
    W_o = np.asarray(W_o, np.float32)
    b_o = np.asarray(b_o, np.float32)
    B = x.shape[0]
    assert x.shape == (8, T, D), f"unexpected x shape {x.shape}"

    if "run" not in _CACHE:
        nc = build_nc()
        _CACHE["run"] = _make_runner(nc, n_cores=8)
    run = _CACHE["run"]

    in_maps = prep_in_maps(x, W_qkv, b_qkv, W_o, b_o)
    res = run(in_maps)
    out = np.stack([res[b]["y"] for b in range(B)]).astype(np.float32)
    return out



# revision 35
# speedup vs baseline: 1.0018x; 1.0011x over previous
"""Multi-head self-attention Bass kernel for TRN2, batch-parallel over 8 cores.

v2: natural-O AV orientation + fused normalize/transpose + pipelined exps.

Per-core problem (batch element b): x [T=1024, D=1024], 16 heads, d_k=64.
Dataflow (trailing T in a name = transposed layout [feature, token]):
  xT   [D, T]       host-pre-transposed input, bf16
  vg   [T, 16*65]   V natural, 65 cols/head: 64 V (strided DVE copy from
                    512-wide psum chunks) + ones col (one global memset)
  qk   [2D, T]      Q^T,K^T tiles, bf16 (written via DVE bias-add from psum)
  ST_h [T_k, T_q]   = K_h Q_h^T  (lhsT=KT_h chunk, rhs=QT_h, K=64)
  ET_h = exp(ST_h/8) bf16, one ACT op per [128, 1024] psum tile
  O_nat[q, 65]      per (pair, q-tile): psum accumulation, lhsT=ET chunk
                    (K=k-tokens), rhs=vg head cols; col 64 = row sums
  o_nat             normalized during DVE copy with per-partition 1/sums
  psT  [128, T]     O^T via identity-matmul transpose of o_nat
  ot copy           adds V bias (per-partition in O^T layout) during copy
  y    [T, D]       = lhsT=ot tile (bf16), rhs=W_o bf16 (+bias via K=1 mm)

Pipelining (window W runs): av(W) q-groups + S(W+1) tk-groups + qkt(W+2)
pieces, interleaved so PE never waits on the ACT exp drain or psum pool
rotation. The V phase is woven with pair-0 S tiles in the prologue.
"""
import numpy as np
import concourse.bacc as bacc
import concourse.mybir as mybir
from concourse.tile import TileContext
from concourse.bass import ts

F32 = mybir.dt.float32
BF16 = mybir.dt.bfloat16
AF = mybir.ActivationFunctionType

T = 1024       # tokens per core (one batch element)
D = 1024       # d_model
H = 16         # heads
DK = 64        # head dim
SCALE = 1.0 / 8.0
NT = T // 128  # 8 token tiles
ND = D // 128  # 8 d tiles
NC_T = T // 512  # 2 free-dim chunks of tokens
HWID = DK + 1      # 65: head V cols + ones col
VW = H * HWID      # 1040, augmented V width
NP = H // 2        # 8 head pairs
ET_BUFS = 5


def build_nc(repeat=1):
    nc = bacc.Bacc(None, target_bir_lowering=False, debug=False)

    xT = nc.dram_tensor("xT", [D, T], BF16, kind="ExternalInput")
    wqk = nc.dram_tensor("wqk", [2 * ND, 128, D], BF16, kind="ExternalInput")
    bqkc = nc.dram_tensor("bqkc", [128, 2 * ND], F32, kind="ExternalInput")
    wv = nc.dram_tensor("wv", [D, D], BF16, kind="ExternalInput")
    bvp = nc.dram_tensor("bvp", [128, NP], F32, kind="ExternalInput")
    wo = nc.dram_tensor("wo", [D, D], BF16, kind="ExternalInput")
    bo = nc.dram_tensor("bo", [1, D], BF16, kind="ExternalInput")
    onesd = nc.dram_tensor("onesd", [1, 128], BF16, kind="ExternalInput")
    identd = nc.dram_tensor("identd", [128, 128], BF16, kind="ExternalInput")
    y = nc.dram_tensor("y", [T, D], F32, kind="ExternalOutput")

    with TileContext(nc) as tc:
      for _rep in range(repeat):
        with (
            tc.tile_pool(name="res", bufs=1) as res,
            tc.tile_pool(name="psW", bufs=3, space="PSUM") as psW,
            tc.tile_pool(name="psO", bufs=2, space="PSUM") as psO,
        ):
            ones_t = res.tile([1, 128], BF16)
            ident_t = res.tile([128, 128], BF16)
            ot = res.tile([128, ND, T], BF16)

            with tc.tile_pool(name="qkvres", bufs=1) as qkvres:
                qk = qkvres.tile([128, 2 * ND, T], BF16)
                vg = qkvres.tile([128, NT, VW], BF16)
                vg_h = vg.rearrange("p t (h w) -> p t h w", w=HWID)

                with (
                    tc.tile_pool(name="xtp", bufs=1) as xtp,
                    tc.tile_pool(name="p2", bufs=1) as p2,
                    tc.tile_pool(name="wqkp", bufs=3) as wqkp,
                    tc.tile_pool(name="etp", bufs=ET_BUFS) as etp,
                    tc.tile_pool(name="onp", bufs=2) as onp,
                    tc.tile_pool(name="invp", bufs=4) as invp,
                ):
                    xt = xtp.tile([128, ND, T], BF16)
                    xT_r = xT.rearrange("(dt p) t -> p dt t", p=128)
                    # xt on Pool+ACT queues; SP stays free for the first wc
                    for d in range(ND):
                        [nc.gpsimd, nc.scalar][d % 2].dma_start(
                            xt[:, d, :], xT_r[:, d, :])
                    # ones columns of vg, written once (V copies skip them)
                    nc.vector.memset(vg_h[:, :, :, DK:DK + 1], 1.0)

                    wqk_r = wqk.rearrange("c p (dt f) -> c p dt f", f=128)
                    wo_r = wo.rearrange("(dt p) c -> p dt c", p=128)

                    def qkt_pieces(c, split_dma=False):
                        """Q^T/K^T c-tile as 4 interleavable pieces."""
                        wc = wqkp.tile([128, ND, 128], BF16, tag="wqk")
                        if split_dma:  # first tiles: halve DMA latency
                            nc.sync.dma_start(wc[:, 0:2, :],
                                              wqk_r[c][:, 0:2, :])
                            nc.sync.dma_start(wc[:, 2:ND, :],
                                              wqk_r[c][:, 2:ND, :])
                        else:
                            nc.sync.dma_start(wc[:], wqk_r[c])
                        state = {}

                        def quarter(i):
                            if i == 0:
                                state["pp"] = psW.tile(
                                    [128, 1024], F32, tag="wide",
                                    name=f"ppqk_{c}")
                            pp = state["pp"]
                            for d in range(i * 2, i * 2 + 2):
                                for tq in range(NC_T):
                                    nc.tensor.matmul(
                                        pp[:, ts(tq, 512)], wc[:, d, :],
                                        xt[:, d, ts(tq, 512)],
                                        start=(d == 0), stop=(d == ND - 1))
                            if i == 3:
                                nc.vector.tensor_scalar_add(
                                    qk[:, c, :], pp[:], bqk_t[:, c:c + 1])
                        return [lambda i=i: quarter(i) for i in range(4)]

                    def s_tile(hp, tk, et_pair):
                        """S matmuls + exp for head pair hp, token tile tk."""
                        for sub in range(2):
                            b0 = sub * 64
                            pss = psW.tile([128, 1024], F32, tag="wide",
                                           name=f"pss_{hp}_{tk}_{sub}")
                            for tq in range(NC_T):
                                nc.tensor.matmul(
                                    pss[:, ts(tq, 512)],
                                    qk[b0:b0 + DK, ND + hp, ts(tk, 128)],
                                    qk[b0:b0 + DK, hp, ts(tq, 512)],
                                    start=True, stop=True,
                                    tile_position=(b0, 0))
                            nc.scalar.activation(
                                et_pair[sub][:, tk, :], pss[:], AF.Exp,
                                scale=SCALE)

                    et_bufs = [
                        [etp.tile([128, NT, T], BF16, tag="et",
                                  name=f"et_{hp}_{s}") for s in range(2)]
                        for hp in range(NP)
                    ]

                    # ---- prologue: V phase woven with pair-0/1 S + qkt ----
                    with tc.tile_pool(name="wvp", bufs=1) as wvp:
                        wvt = wvp.tile([128, ND, D], BF16)
                        wv_r = wv.rearrange("(dt p) c -> p dt c", p=128)
                        pieces01 = (qkt_pieces(0, split_dma=True) +
                                    qkt_pieces(ND, split_dma=True))
                        for d in range(ND):
                            [nc.gpsimd, nc.scalar][d % 2].dma_start(
                                wvt[:, d, :], wv_r[:, d, :])
                        bqk_t = p2.tile([128, 2 * ND], F32)
                        nc.sync.dma_start(bqk_t[:], bqkc[:])
                        bvp_t = p2.tile([128, NP], F32)
                        nc.sync.dma_start(bvp_t[:], bvp[:])
                        bo_t = qkvres.tile([1, D], BF16)
                        nc.sync.dma_start(bo_t[:], bo[:])
                        nc.sync.dma_start(ones_t[:], onesd[:])
                        nc.sync.dma_start(ident_t[:], identd[:])
                        for f in pieces01:
                            f()
                        extra = qkt_pieces(1) + qkt_pieces(ND + 1)
                        for t in range(NT):
                            if t > 0:  # S shifted one slot: V(0) covers the
                                s_tile(0, t - 1, et_bufs[0])  # qk bias-add
                            # V projection for token tile t: two 512 chunks
                            for ch in range(2):
                                pp = psW.tile([128, 1024], F32, tag="wide")
                                for d in range(ND):
                                    nc.tensor.matmul(
                                        pp[:, :512], xt[:, d, ts(t, 128)],
                                        wvt[:, d, ts(ch, 512)],
                                        start=(d == 0), stop=(d == ND - 1))
                                pp_h = pp.rearrange("p (h w) -> p h w", w=DK)
                                nc.vector.tensor_copy(
                                    vg_h[:, t, ch * 8:ch * 8 + 8, 0:DK],
                                    pp_h[:, 0:8, :])
                            if extra[t] is not None:
                                extra[t]()
                        s_tile(0, NT - 1, et_bufs[0])

                    # broadcast bias tile: y bias via DVE add, not mms
                    bo_bc = qkvres.tile([128, D], BF16)
                    pbps = psW.tile([128, 1024], F32, tag="wide",
                                    name="pb_bias")
                    for oc in range(NC_T):
                        nc.tensor.matmul(pbps[:, ts(oc, 512)], ones_t[:],
                                         bo_t[:, ts(oc, 512)],
                                         start=True, stop=True)
                    nc.vector.tensor_copy(bo_bc[:], pbps[:])

                    # ---- steady windows W: av(W) | S(W+1) | qkt(W+2) ----
                    wo_t = qkvres.tile([128, ND, D], BF16)
                    py_tiles = {}

                    def y_part(t, hi):
                        if t not in py_tiles:
                            py_tiles[t] = [psW.tile([128, 1024], F32,
                                                    tag="wide",
                                                    name=f"py_{t}"), 0]
                        ent = py_tiles[t]
                        for oc in range(NC_T):
                            for d in range(ent[1], hi):
                                nc.tensor.matmul(
                                    ent[0][:, ts(oc, 512)],
                                    ot[:, d, ts(t, 128)],
                                    wo_t[:, d, ts(oc, 512)],
                                    start=(d == 0), stop=False)
                        ent[1] = hi

                    def av_q(W, q, on_t):
                        et_pair = et_bufs[W]
                        po = psO.tile([128, 2 * HWID], F32, tag="po",
                                      name=f"po_{W}_{q}")
                        for sub in range(2):
                            h = 2 * W + sub
                            for kt in range(NT):
                                nc.tensor.matmul(
                                    po[:, sub * HWID:(sub + 1) * HWID],
                                    et_pair[sub][:, kt, ts(q, 128)],
                                    vg[:, kt, h * HWID:(h + 1) * HWID],
                                    start=(kt == 0), stop=(kt == NT - 1))
                        iv = invp.tile([128, 2], F32, tag="inv",
                                       name=f"inv_{W}_{q}")
                        for sub in range(2):
                            nc.vector.reciprocal(
                                iv[:, sub:sub + 1],
                                po[:, sub * HWID + DK:sub * HWID + DK + 1])
                        for sub in range(2):
                            nc.vector.tensor_scalar_mul(
                                on_t[:, q, sub * DK:(sub + 1) * DK],
                                po[:, sub * HWID:sub * HWID + DK],
                                iv[:, sub:sub + 1])

                    def trans_half(W, on_t, st, half):
                        if half == 0:
                            st["psT"] = psW.tile([128, 1024], F32,
                                                 tag="wide", name=f"psT_{W}")
                        psT = st["psT"]
                        for q in range(half * 4, half * 4 + 4):
                            nc.tensor.matmul(
                                psT[:, ts(q, 128)], on_t[:, q, :],
                                ident_t[:], start=True, stop=True)
                        if half == 1:
                            for hh in range(2):  # halves: y's d-mm for the
                                # first token chunks unblocks sooner
                                nc.vector.tensor_scalar_add(
                                    ot[:, W, ts(hh, 512)],
                                    psT[:, ts(hh, 512)], bvp_t[:, W:W + 1])

                    for W in range(NP):
                        on_t = onp.tile([128, NT, 128], BF16, tag="onat",
                                        name=f"onat_{W}")
                        s_p = ([(lambda tk=tk:
                                 s_tile(W + 1, tk, et_bufs[W + 1]))
                                for tk in range(NT)] if W + 1 < NP
                               else [None] * NT)
                        q_p = (qkt_pieces(W + 2) + qkt_pieces(ND + W + 2)
                               if W + 2 < NP else [None] * 8)
                        av = [lambda q=q, W=W, o=on_t: av_q(W, q, o)
                              for q in range(NT)]
                        st = {}
                        trA = (lambda W=W, o=on_t, s=st:
                               trans_half(W, o, s, 0))
                        trB = (lambda W=W, o=on_t, s=st:
                               trans_half(W, o, s, 1))
                        if W < NP - 2:
                            order = [q_p[0], s_p[0], q_p[1], av[0],
                                     s_p[1], q_p[2], av[1], s_p[2],
                                     q_p[3], av[2], s_p[3], q_p[4],
                                     av[3], s_p[4], q_p[5], av[4],
                                     s_p[5], q_p[6], av[5], s_p[6],
                                     q_p[7], av[6], s_p[7], av[7],
                                     trA, trB]
                        elif W == NP - 2:
                            # no more qkt work: fill with partial y groups
                            # (d0..d5 need only pairs 0-5); tks 0-1 of S(7)
                            # were already issued at window 5's end
                            order = [s_p[2], av[0], s_p[3], av[1],
                                     s_p[4], av[2], av[3],
                                     lambda: y_part(0, 3),
                                     s_p[5], av[4],
                                     lambda: y_part(0, 6),
                                     s_p[6], av[5],
                                     s_p[7], av[6], av[7], trA, trB]
                        else:
                            # last pair: fill exp-wait stalls with partial
                            # y-projection groups (d<=6 need pairs 0-6 only)
                            order = [av[0], av[1], av[2],
                                     lambda: y_part(1, 3),
                                     av[3], lambda: y_part(1, 6),
                                     av[4], lambda: y_part(0, 7),
                                     av[5], av[6],
                                     lambda: y_part(1, 7),
                                     av[7], trA, trB]
                        for f in order:
                            if f is not None:
                                f()
                        if W == NP - 3:  # prefetch W_o during tail pairs
                            for d in range(ND):
                                nc.gpsimd.dma_start(wo_t[:, d, :],
                                                    wo_r[:, d, :])
                            # feed ACT across the boundary into window 6
                            s_tile(NP - 1, 0, et_bufs[NP - 1])
                            s_tile(NP - 1, 1, et_bufs[NP - 1])

                # ---- output projection (all bf16) ----
                with tc.tile_pool(name="yp", bufs=4) as yp:
                    # t=2 first, from the freed psO banks: its d0..d6 mms
                    # cover the trans(7)->ot copy latency while the wide
                    # pool is still pinned by the window-7 partials
                    p2a = psO.tile([128, 512], F32, tag="po", name="p2a")
                    p2b = psO.tile([128, 512], F32, tag="po", name="p2b")
                    for oc, pyx in ((0, p2a), (1, p2b)):
                        for d in range(ND - 1):
                            nc.tensor.matmul(
                                pyx[:], ot[:, d, ts(2, 128)],
                                wo_t[:, d, ts(oc, 512)],
                                start=(d == 0), stop=False)
                    for t in [2, 0, 1, 3, 4, 5, 6, 7]:
                        if t == 2:
                            grp = [(0, p2a, ND - 1), (1, p2b, ND - 1)]
                        elif t in py_tiles:
                            py, d0 = py_tiles[t]
                            grp = [(oc, py[:, ts(oc, 512)], d0)
                                   for oc in range(NC_T)]
                        else:
                            py = psW.tile([128, 1024], F32, tag="wide",
                                          name=f"pyf_{t}")
                            grp = [(oc, py[:, ts(oc, 512)], 0)
                                   for oc in range(NC_T)]
                        for oc, pyx, d0 in grp:
                            last = (t == NT - 1 and oc == NC_T - 1)
                            use_act = (t % 2 == 1) and not last
                            for d in range(d0, ND):
                                nc.tensor.matmul(
                                    pyx[:], ot[:, d, ts(t, 128)],
                                    wo_t[:, d, ts(oc, 512)],
                                    start=(d == 0),
                                    stop=(d == ND - 1) and not use_act)
                            yt = yp.tile([128, 512], F32, tag="yt")
                            if use_act:  # bias via mm, copy on idle ACT
                                nc.tensor.matmul(
                                    pyx[:], ones_t[:], bo_t[:, ts(oc, 512)],
                                    start=False, stop=True)
                                nc.scalar.copy(yt[:], pyx[:])
                                nc.sync.dma_start(
                                    y[ts(t, 128), ts(oc, 512)], yt[:])
                            elif last:
                                # two pieces, two DMA queues: parallel
                                # completion chains shorten the drain
                                for hh in range(2):
                                    nc.vector.tensor_add(
                                        yt[:, ts(hh, 256)],
                                        pyx[:, ts(hh, 256)],
                                        bo_bc[:, oc * 512 + hh * 256:
                                              oc * 512 + hh * 256 + 256])
                                    [nc.sync, nc.scalar][hh].dma_start(
                                        y[ts(t, 128), oc * 512 + hh * 256:
                                          oc * 512 + hh * 256 + 256],
                                        yt[:, ts(hh, 256)])
                            else:
                                nc.vector.tensor_add(yt[:], pyx[:],
                                                     bo_bc[:, ts(oc, 512)])
                                nc.sync.dma_start(
                                    y[ts(t, 128), ts(oc, 512)], yt[:])

    nc.finalize()
    return nc


def prep_in_maps(x, W_qkv, b_qkv, W_o, b_o):
    """Host-side sharding: batch-parallel, one batch element per core."""
    import ml_dtypes
    bf16 = ml_dtypes.bfloat16
    B = x.shape[0]
    # wqk relayout: [2ND c-tiles, 128 partitions(cols), D contiguous]
    W_qk = np.asarray(W_qkv[:, :2 * D], np.float32)
    wqk_t = np.transpose(
        W_qk.reshape(ND, 128, 2 * ND, 128), (2, 1, 0, 3)
    ).reshape(2 * ND, 128, D)  # [c, row-in-dtile(p), d-tile*128+col]
    b_qkc = np.ascontiguousarray(
        np.asarray(b_qkv[:2 * D], np.float32).reshape(2 * ND, 128).T)
    W_vo = np.ascontiguousarray(W_qkv[:, 2 * D:])    # [D, D] V weights
    b_vo = np.asarray(b_qkv[2 * D:], np.float32)
    bvp_a = np.ascontiguousarray(b_vo.reshape(NP, 128).T)
    ones = np.ones((1, 128), bf16)
    ident = np.eye(128, dtype=np.float32).astype(bf16)
    in_maps = []
    for b in range(B):
        in_maps.append({
            "xT": np.ascontiguousarray(x[b].T).astype(bf16),
            "wqk": np.ascontiguousarray(wqk_t).astype(bf16),
            "bqkc": b_qkc, "bvp": bvp_a,
            "wv": W_vo.astype(bf16),
            "wo": np.ascontiguousarray(W_o).astype(bf16),
            "bo": np.ascontiguousarray(b_o).reshape(1, -1).astype(bf16),
            "onesd": ones, "identd": ident,
        })
    return in_maps


# ---------------------------------------------------------------------------
# Self-contained SPMD runner (axon PJRT path) and the graded entry point.
# ---------------------------------------------------------------------------
import jax as _jax


_CACHE = {}


def _make_runner(nc, n_cores=8):
    from jax.sharding import Mesh, PartitionSpec
    from jax.experimental.shard_map import shard_map
    from concourse import bass2jax

    bass2jax.install_neuronx_cc_hook()
    partition_name = nc.partition_id_tensor.name if nc.partition_id_tensor else None
    in_names, out_names, out_avals, zero_outs = [], [], [], []
    for alloc in nc.m.functions[0].allocations:
        if not isinstance(alloc, mybir.MemoryLocationSet):
            continue
        name = alloc.memorylocations[0].name
        if alloc.kind == "ExternalInput":
            if name != partition_name:
                in_names.append(name)
        elif alloc.kind == "ExternalOutput":
            shape = tuple(alloc.tensor_shape)
            dtype = mybir.dt.np(alloc.dtype)
            out_names.append(name)
            out_avals.append(_jax.core.ShapedArray(shape, dtype))
            zero_outs.append(np.zeros(shape, dtype))
    n_params = len(in_names)
    all_in_names = list(in_names) + list(out_names)
    if partition_name is not None:
        all_in_names.append(partition_name)

    def _body(*args):
        operands = list(args)
        if partition_name is not None:
            operands.append(bass2jax.partition_id_tensor())
        return tuple(bass2jax._bass_exec_p.bind(
            *operands,
            out_avals=tuple(out_avals),
            in_names=tuple(all_in_names),
            out_names=tuple(out_names),
            lowering_input_output_aliases=(),
            sim_require_finite=True,
            sim_require_nnan=True,
            nc=nc,
        ))

    devices = _jax.devices()[:n_cores]
    mesh = Mesh(np.asarray(devices), ("core",))
    nin = n_params + len(out_names)
    sharded = _jax.jit(
        shard_map(_body, mesh=mesh,
                  in_specs=(PartitionSpec("core"),) * nin,
                  out_specs=(PartitionSpec("core"),) * len(out_names),
                  check_rep=False),
        keep_unused=True,
    )

    def run(in_maps):
        concat_in = [
            np.concatenate([np.asarray(m[name]) for m in in_maps], axis=0)
            for name in in_names
        ]
        concat_zeros = [
            np.zeros((n_cores * z.shape[0], *z.shape[1:]), z.dtype)
            for z in zero_outs
        ]
        out_arrs = [np.asarray(o) for o in sharded(*concat_in, *concat_zeros)]
        return [
            {name: out_arrs[i].reshape(n_cores, *out_avals[i].shape)[c]
             for i, name in enumerate(out_names)}
            for c in range(n_cores)
        ]

    return run


def kernel(x, W_qkv, b_qkv, W_o, b_o):
    """Full-input entry point: shards batch across the 8 NeuronCores,
    runs the Bass MHA kernel SPMD, gathers the full output."""
    x = np.ascontiguousarray(np.asarray(x, np.float32))
    W_qkv = np.asarray(W_qkv, np.float32)
    b_qkv = np.asarray(b_qkv, np.float32)
    W_o = np.asarray(W_o, np.float32)
    b_o = np.asarray(b_o, np.float32)
    B = x.shape[0]
    assert x.shape == (8, T, D), f"unexpected x shape {x.shape}"

    if "run" not in _CACHE:
        nc = build_nc()
        _CACHE["run"] = _make_runner(nc, n_cores=8)
    run = _CACHE["run"]

    in_maps = prep_in_maps(x, W_qkv, b_qkv, W_o, b_o)
    res = run(in_maps)
    out = np.stack([res[b]["y"] for b in range(B)]).astype(np.float32)
    return out



# revision 36
# speedup vs baseline: 1.0038x; 1.0021x over previous
"""Multi-head self-attention Bass kernel for TRN2, batch-parallel over 8 cores.

v2: natural-O AV orientation + fused normalize/transpose + pipelined exps.

Per-core problem (batch element b): x [T=1024, D=1024], 16 heads, d_k=64.
Dataflow (trailing T in a name = transposed layout [feature, token]):
  xT   [D, T]       host-pre-transposed input, bf16
  vg   [T, 16*65]   V natural, 65 cols/head: 64 V (strided DVE copy from
                    512-wide psum chunks) + ones col (one global memset)
  qk   [2D, T]      Q^T,K^T tiles, bf16 (written via DVE bias-add from psum)
  ST_h [T_k, T_q]   = K_h Q_h^T  (lhsT=KT_h chunk, rhs=QT_h, K=64)
  ET_h = exp(ST_h/8) bf16, one ACT op per [128, 1024] psum tile
  O_nat[q, 65]      per (pair, q-tile): psum accumulation, lhsT=ET chunk
                    (K=k-tokens), rhs=vg head cols; col 64 = row sums
  o_nat             normalized during DVE copy with per-partition 1/sums
  psT  [128, T]     O^T via identity-matmul transpose of o_nat
  ot copy           adds V bias (per-partition in O^T layout) during copy
  y    [T, D]       = lhsT=ot tile (bf16), rhs=W_o bf16 (+bias via K=1 mm)

Pipelining (window W runs): av(W) q-groups + S(W+1) tk-groups + qkt(W+2)
pieces, interleaved so PE never waits on the ACT exp drain or psum pool
rotation. The V phase is woven with pair-0 S tiles in the prologue.
"""
import numpy as np
import concourse.bacc as bacc
import concourse.mybir as mybir
from concourse.tile import TileContext
from concourse.bass import ts

F32 = mybir.dt.float32
BF16 = mybir.dt.bfloat16
AF = mybir.ActivationFunctionType

T = 1024       # tokens per core (one batch element)
D = 1024       # d_model
H = 16         # heads
DK = 64        # head dim
SCALE = 1.0 / 8.0
NT = T // 128  # 8 token tiles
ND = D // 128  # 8 d tiles
NC_T = T // 512  # 2 free-dim chunks of tokens
HWID = DK + 1      # 65: head V cols + ones col
VW = H * HWID      # 1040, augmented V width
NP = H // 2        # 8 head pairs
ET_BUFS = 5


def build_nc(repeat=1):
    nc = bacc.Bacc(None, target_bir_lowering=False, debug=False)

    xT = nc.dram_tensor("xT", [D, T], BF16, kind="ExternalInput")
    wqk = nc.dram_tensor("wqk", [2 * ND, 128, D], BF16, kind="ExternalInput")
    bqkc = nc.dram_tensor("bqkc", [128, 2 * ND], F32, kind="ExternalInput")
    wv = nc.dram_tensor("wv", [D, D], BF16, kind="ExternalInput")
    bvp = nc.dram_tensor("bvp", [128, NP], F32, kind="ExternalInput")
    wo = nc.dram_tensor("wo", [D, D], BF16, kind="ExternalInput")
    bo = nc.dram_tensor("bo", [1, D], BF16, kind="ExternalInput")
    onesd = nc.dram_tensor("onesd", [1, 128], BF16, kind="ExternalInput")
    identd = nc.dram_tensor("identd", [128, 128], BF16, kind="ExternalInput")
    y = nc.dram_tensor("y", [T, D], F32, kind="ExternalOutput")

    with TileContext(nc) as tc:
      for _rep in range(repeat):
        with (
            tc.tile_pool(name="res", bufs=1) as res,
            tc.tile_pool(name="psW", bufs=3, space="PSUM") as psW,
            tc.tile_pool(name="psO", bufs=2, space="PSUM") as psO,
        ):
            ones_t = res.tile([1, 128], BF16)
            ident_t = res.tile([128, 128], BF16)
            ot = res.tile([128, ND, T], BF16)

            with tc.tile_pool(name="qkvres", bufs=1) as qkvres:
                qk = qkvres.tile([128, 2 * ND, T], BF16)
                vg = qkvres.tile([128, NT, VW], BF16)
                vg_h = vg.rearrange("p t (h w) -> p t h w", w=HWID)

                with (
                    tc.tile_pool(name="xtp", bufs=1) as xtp,
                    tc.tile_pool(name="p2", bufs=1) as p2,
                    tc.tile_pool(name="wqkp", bufs=3) as wqkp,
                    tc.tile_pool(name="etp", bufs=ET_BUFS) as etp,
                    tc.tile_pool(name="onp", bufs=2) as onp,
                    tc.tile_pool(name="invp", bufs=4) as invp,
                ):
                    xt = xtp.tile([128, ND, T], BF16)
                    xT_r = xT.rearrange("(dt p) t -> p dt t", p=128)
                    # xt on Pool+ACT queues; SP stays free for the first wc
                    for d in range(ND):
                        [nc.gpsimd, nc.scalar][d % 2].dma_start(
                            xt[:, d, :], xT_r[:, d, :])
                    # ones columns of vg, written once (V copies skip them)
                    nc.vector.memset(vg_h[:, :, :, DK:DK + 1], 1.0)

                    wqk_r = wqk.rearrange("c p (dt f) -> c p dt f", f=128)
                    wo_r = wo.rearrange("(dt p) c -> p dt c", p=128)

                    def qkt_pieces(c, split_dma=False):
                        """Q^T/K^T c-tile as 4 interleavable pieces."""
                        wc = wqkp.tile([128, ND, 128], BF16, tag="wqk")
                        if split_dma:  # first tiles: halve DMA latency
                            nc.sync.dma_start(wc[:, 0:2, :],
                                              wqk_r[c][:, 0:2, :])
                            nc.sync.dma_start(wc[:, 2:ND, :],
                                              wqk_r[c][:, 2:ND, :])
                        else:
                            nc.sync.dma_start(wc[:], wqk_r[c])
                        state = {}

                        def quarter(i):
                            if i == 0:
                                state["pp"] = psW.tile(
                                    [128, 1024], F32, tag="wide",
                                    name=f"ppqk_{c}")
                            pp = state["pp"]
                            for d in range(i * 2, i * 2 + 2):
                                for tq in range(NC_T):
                                    nc.tensor.matmul(
                                        pp[:, ts(tq, 512)], wc[:, d, :],
                                        xt[:, d, ts(tq, 512)],
                                        start=(d == 0), stop=(d == ND - 1))
                            if i == 3:
                                nc.vector.tensor_scalar_add(
                                    qk[:, c, :], pp[:], bqk_t[:, c:c + 1])
                        return [lambda i=i: quarter(i) for i in range(4)]

                    def s_tile(hp, tk, et_pair):
                        """S matmuls + exp for head pair hp, token tile tk."""
                        for sub in range(2):
                            b0 = sub * 64
                            pss = psW.tile([128, 1024], F32, tag="wide",
                                           name=f"pss_{hp}_{tk}_{sub}")
                            for tq in range(NC_T):
                                nc.tensor.matmul(
                                    pss[:, ts(tq, 512)],
                                    qk[b0:b0 + DK, ND + hp, ts(tk, 128)],
                                    qk[b0:b0 + DK, hp, ts(tq, 512)],
                                    start=True, stop=True,
                                    tile_position=(b0, 0))
                            nc.scalar.activation(
                                et_pair[sub][:, tk, :], pss[:], AF.Exp,
                                scale=SCALE)

                    et_bufs = [
                        [etp.tile([128, NT, T], BF16, tag="et",
                                  name=f"et_{hp}_{s}") for s in range(2)]
                        for hp in range(NP)
                    ]

                    # ---- prologue: V phase woven with pair-0/1 S + qkt ----
                    with tc.tile_pool(name="wvp", bufs=1) as wvp:
                        wvt = wvp.tile([128, ND, D], BF16)
                        wv_r = wv.rearrange("(dt p) c -> p dt c", p=128)
                        pieces01 = (qkt_pieces(0, split_dma=True) +
                                    qkt_pieces(ND, split_dma=True))
                        for d in range(ND):
                            [nc.gpsimd, nc.scalar][d % 2].dma_start(
                                wvt[:, d, :], wv_r[:, d, :])
                        bqk_t = p2.tile([128, 2 * ND], F32)
                        nc.sync.dma_start(bqk_t[:], bqkc[:])
                        bvp_t = p2.tile([128, NP], F32)
                        nc.sync.dma_start(bvp_t[:], bvp[:])
                        bo_t = qkvres.tile([1, D], BF16)
                        nc.sync.dma_start(bo_t[:], bo[:])
                        nc.sync.dma_start(ones_t[:], onesd[:])
                        nc.sync.dma_start(ident_t[:], identd[:])
                        for f in pieces01:
                            f()
                        extra = qkt_pieces(1) + qkt_pieces(ND + 1)
                        for t in range(NT):
                            if t > 0:  # S shifted one slot: V(0) covers the
                                s_tile(0, t - 1, et_bufs[0])  # qk bias-add
                            # V projection for token tile t: two 512 chunks
                            for ch in range(2):
                                pp = psW.tile([128, 1024], F32, tag="wide")
                                for d in range(ND):
                                    nc.tensor.matmul(
                                        pp[:, :512], xt[:, d, ts(t, 128)],
                                        wvt[:, d, ts(ch, 512)],
                                        start=(d == 0), stop=(d == ND - 1))
                                pp_h = pp.rearrange("p (h w) -> p h w", w=DK)
                                nc.vector.tensor_copy(
                                    vg_h[:, t, ch * 8:ch * 8 + 8, 0:DK],
                                    pp_h[:, 0:8, :])
                            if extra[t] is not None:
                                extra[t]()
                        s_tile(0, NT - 1, et_bufs[0])

                    # broadcast bias tile: y bias via DVE add, not mms
                    bo_bc = qkvres.tile([128, D], BF16)
                    pbps = psW.tile([128, 1024], F32, tag="wide",
                                    name="pb_bias")
                    for oc in range(NC_T):
                        nc.tensor.matmul(pbps[:, ts(oc, 512)], ones_t[:],
                                         bo_t[:, ts(oc, 512)],
                                         start=True, stop=True)
                    nc.vector.tensor_copy(bo_bc[:], pbps[:])

                    # ---- steady windows W: av(W) | S(W+1) | qkt(W+2) ----
                    wo_t = qkvres.tile([128, ND, D], BF16)
                    py_tiles = {}

                    def y_part(t, hi):
                        if t not in py_tiles:
                            py_tiles[t] = [psW.tile([128, 1024], F32,
                                                    tag="wide",
                                                    name=f"py_{t}"), 0]
                        ent = py_tiles[t]
                        for oc in range(NC_T):
                            for d in range(ent[1], hi):
                                nc.tensor.matmul(
                                    ent[0][:, ts(oc, 512)],
                                    ot[:, d, ts(t, 128)],
                                    wo_t[:, d, ts(oc, 512)],
                                    start=(d == 0), stop=False)
                        ent[1] = hi

                    def av_q(W, q, on_t):
                        et_pair = et_bufs[W]
                        po = psO.tile([128, 2 * HWID], F32, tag="po",
                                      name=f"po_{W}_{q}")
                        for sub in range(2):
                            h = 2 * W + sub
                            for kt in range(NT):
                                nc.tensor.matmul(
                                    po[:, sub * HWID:(sub + 1) * HWID],
                                    et_pair[sub][:, kt, ts(q, 128)],
                                    vg[:, kt, h * HWID:(h + 1) * HWID],
                                    start=(kt == 0), stop=(kt == NT - 1))
                        iv = invp.tile([128, 2], F32, tag="inv",
                                       name=f"inv_{W}_{q}")
                        for sub in range(2):
                            nc.vector.reciprocal(
                                iv[:, sub:sub + 1],
                                po[:, sub * HWID + DK:sub * HWID + DK + 1])
                        for sub in range(2):
                            nc.vector.tensor_scalar_mul(
                                on_t[:, q, sub * DK:(sub + 1) * DK],
                                po[:, sub * HWID:sub * HWID + DK],
                                iv[:, sub:sub + 1])

                    def trans_half(W, on_t, st, half):
                        if half == 0:
                            st["psT"] = psW.tile([128, 1024], F32,
                                                 tag="wide", name=f"psT_{W}")
                        psT = st["psT"]
                        for q in range(half * 4, half * 4 + 4):
                            nc.tensor.matmul(
                                psT[:, ts(q, 128)], on_t[:, q, :],
                                ident_t[:], start=True, stop=True)
                        if half == 1:
                            for hh in range(2):  # halves: y's d-mm for the
                                # first token chunks unblocks sooner
                                nc.vector.tensor_scalar_add(
                                    ot[:, W, ts(hh, 512)],
                                    psT[:, ts(hh, 512)], bvp_t[:, W:W + 1])

                    for W in range(NP):
                        on_t = onp.tile([128, NT, 128], BF16, tag="onat",
                                        name=f"onat_{W}")
                        s_p = ([(lambda tk=tk:
                                 s_tile(W + 1, tk, et_bufs[W + 1]))
                                for tk in range(NT)] if W + 1 < NP
                               else [None] * NT)
                        q_p = (qkt_pieces(W + 2) + qkt_pieces(ND + W + 2)
                               if W + 2 < NP else [None] * 8)
                        av = [lambda q=q, W=W, o=on_t: av_q(W, q, o)
                              for q in range(NT)]
                        st = {}
                        trA = (lambda W=W, o=on_t, s=st:
                               trans_half(W, o, s, 0))
                        trB = (lambda W=W, o=on_t, s=st:
                               trans_half(W, o, s, 1))
                        if W < NP - 2:
                            order = [q_p[0], s_p[0], q_p[1], av[0],
                                     s_p[1], q_p[2], av[1], s_p[2],
                                     q_p[3], av[2], s_p[3], q_p[4],
                                     av[3], s_p[4], q_p[5], av[4],
                                     s_p[5], q_p[6], av[5], s_p[6],
                                     q_p[7], av[6], s_p[7], av[7],
                                     trA, trB]
                        elif W == NP - 2:
                            # no more qkt work: fill with partial y groups
                            # (d0..d5 need only pairs 0-5); tks 0-1 of S(7)
                            # were already issued at window 5's end
                            order = [s_p[2], av[0], s_p[3], av[1],
                                     s_p[4], av[2], av[3],
                                     lambda: y_part(0, 3),
                                     s_p[5], av[4],
                                     lambda: y_part(0, 6),
                                     s_p[6], av[5],
                                     s_p[7], av[6], av[7], trA, trB]
                        else:
                            # last pair: fill exp-wait stalls with partial
                            # y-projection groups (d<=6 need pairs 0-6 only)
                            order = [av[0], av[1], av[2],
                                     lambda: y_part(1, 3),
                                     av[3], lambda: y_part(1, 6),
                                     av[4], lambda: y_part(0, 7),
                                     av[5], av[6],
                                     lambda: y_part(1, 7),
                                     av[7], trA, trB]
                        for f in order:
                            if f is not None:
                                f()
                        if W == NP - 3:  # prefetch W_o during tail pairs
                            for d in range(ND):
                                nc.gpsimd.dma_start(wo_t[:, d, :],
                                                    wo_r[:, d, :])
                            # feed ACT across the boundary into window 6
                            s_tile(NP - 1, 0, et_bufs[NP - 1])
                            s_tile(NP - 1, 1, et_bufs[NP - 1])

                # ---- output projection (all bf16) ----
                with tc.tile_pool(name="yp", bufs=4) as yp:
                    # t=2 first, from the freed psO banks: its d0..d6 mms
                    # cover the trans(7)->ot copy latency while the wide
                    # pool is still pinned by the window-7 partials
                    p2a = psO.tile([128, 512], F32, tag="po", name="p2a")
                    p2b = psO.tile([128, 512], F32, tag="po", name="p2b")
                    for oc, pyx in ((0, p2a), (1, p2b)):
                        for d in range(ND - 1):
                            nc.tensor.matmul(
                                pyx[:], ot[:, d, ts(2, 128)],
                                wo_t[:, d, ts(oc, 512)],
                                start=(d == 0), stop=False)
                    for t in [2, 0, 1, 3, 4, 5, 6, 7]:
                        if t == 5:  # reuse psO banks freed by t=2
                            p5a = psO.tile([128, 512], F32, tag="po",
                                           name="p5a")
                            p5b = psO.tile([128, 512], F32, tag="po",
                                           name="p5b")
                            grp = [(0, p5a, 0), (1, p5b, 0)]
                        elif t == 2:
                            grp = [(0, p2a, ND - 1), (1, p2b, ND - 1)]
                        elif t in py_tiles:
                            py, d0 = py_tiles[t]
                            grp = [(oc, py[:, ts(oc, 512)], d0)
                                   for oc in range(NC_T)]
                        else:
                            py = psW.tile([128, 1024], F32, tag="wide",
                                          name=f"pyf_{t}")
                            grp = [(oc, py[:, ts(oc, 512)], 0)
                                   for oc in range(NC_T)]
                        for oc, pyx, d0 in grp:
                            last = (t == NT - 1 and oc == NC_T - 1)
                            use_act = (t % 2 == 1) and not last
                            for d in range(d0, ND):
                                nc.tensor.matmul(
                                    pyx[:], ot[:, d, ts(t, 128)],
                                    wo_t[:, d, ts(oc, 512)],
                                    start=(d == 0),
                                    stop=(d == ND - 1) and not use_act)
                            yt = yp.tile([128, 512], F32, tag="yt")
                            if use_act:  # bias via mm, copy on idle ACT
                                nc.tensor.matmul(
                                    pyx[:], ones_t[:], bo_t[:, ts(oc, 512)],
                                    start=False, stop=True)
                                nc.scalar.copy(yt[:], pyx[:])
                                nc.sync.dma_start(
                                    y[ts(t, 128), ts(oc, 512)], yt[:])
                            elif last:
                                # two pieces, two DMA queues: parallel
                                # completion chains shorten the drain
                                for hh in range(2):
                                    nc.vector.tensor_add(
                                        yt[:, ts(hh, 256)],
                                        pyx[:, ts(hh, 256)],
                                        bo_bc[:, oc * 512 + hh * 256:
                                              oc * 512 + hh * 256 + 256])
                                    [nc.sync, nc.scalar][hh].dma_start(
                                        y[ts(t, 128), oc * 512 + hh * 256:
                                          oc * 512 + hh * 256 + 256],
                                        yt[:, ts(hh, 256)])
                            else:
                                nc.vector.tensor_add(yt[:], pyx[:],
                                                     bo_bc[:, ts(oc, 512)])
                                nc.sync.dma_start(
                                    y[ts(t, 128), ts(oc, 512)], yt[:])

    nc.finalize()
    return nc


def prep_in_maps(x, W_qkv, b_qkv, W_o, b_o):
    """Host-side sharding: batch-parallel, one batch element per core."""
    import ml_dtypes
    bf16 = ml_dtypes.bfloat16
    B = x.shape[0]
    # wqk relayout: [2ND c-tiles, 128 partitions(cols), D contiguous]
    W_qk = np.asarray(W_qkv[:, :2 * D], np.float32)
    wqk_t = np.transpose(
        W_qk.reshape(ND, 128, 2 * ND, 128), (2, 1, 0, 3)
    ).reshape(2 * ND, 128, D)  # [c, row-in-dtile(p), d-tile*128+col]
    b_qkc = np.ascontiguousarray(
        np.asarray(b_qkv[:2 * D], np.float32).reshape(2 * ND, 128).T)
    W_vo = np.ascontiguousarray(W_qkv[:, 2 * D:])    # [D, D] V weights
    b_vo = np.asarray(b_qkv[2 * D:], np.float32)
    bvp_a = np.ascontiguousarray(b_vo.reshape(NP, 128).T)
    ones = np.ones((1, 128), bf16)
    ident = np.eye(128, dtype=np.float32).astype(bf16)
    in_maps = []
    for b in range(B):
        in_maps.append({
            "xT": np.ascontiguousarray(x[b].T).astype(bf16),
            "wqk": np.ascontiguousarray(wqk_t).astype(bf16),
            "bqkc": b_qkc, "bvp": bvp_a,
            "wv": W_vo.astype(bf16),
            "wo": np.ascontiguousarray(W_o).astype(bf16),
            "bo": np.ascontiguousarray(b_o).reshape(1, -1).astype(bf16),
            "onesd": ones, "identd": ident,
        })
    return in_maps


# ---------------------------------------------------------------------------
# Self-contained SPMD runner (axon PJRT path) and the graded entry point.
# ---------------------------------------------------------------------------
import jax as _jax


_CACHE = {}


def _make_runner(nc, n_cores=8):
    from jax.sharding import Mesh, PartitionSpec
    from jax.experimental.shard_map import shard_map
    from concourse import bass2jax

    bass2jax.install_neuronx_cc_hook()
    partition_name = nc.partition_id_tensor.name if nc.partition_id_tensor else None
    in_names, out_names, out_avals, zero_outs = [], [], [], []
    for alloc in nc.m.functions[0].allocations:
        if not isinstance(alloc, mybir.MemoryLocationSet):
            continue
        name = alloc.memorylocations[0].name
        if alloc.kind == "ExternalInput":
            if name != partition_name:
                in_names.append(name)
        elif alloc.kind == "ExternalOutput":
            shape = tuple(alloc.tensor_shape)
            dtype = mybir.dt.np(alloc.dtype)
            out_names.append(name)
            out_avals.append(_jax.core.ShapedArray(shape, dtype))
            zero_outs.append(np.zeros(shape, dtype))
    n_params = len(in_names)
    all_in_names = list(in_names) + list(out_names)
    if partition_name is not None:
        all_in_names.append(partition_name)

    def _body(*args):
        operands = list(args)
        if partition_name is not None:
            operands.append(bass2jax.partition_id_tensor())
        return tuple(bass2jax._bass_exec_p.bind(
            *operands,
            out_avals=tuple(out_avals),
            in_names=tuple(all_in_names),
            out_names=tuple(out_names),
            lowering_input_output_aliases=(),
            sim_require_finite=True,
            sim_require_nnan=True,
            nc=nc,
        ))

    devices = _jax.devices()[:n_cores]
    mesh = Mesh(np.asarray(devices), ("core",))
    nin = n_params + len(out_names)
    sharded = _jax.jit(
        shard_map(_body, mesh=mesh,
                  in_specs=(PartitionSpec("core"),) * nin,
                  out_specs=(PartitionSpec("core"),) * len(out_names),
                  check_rep=False),
        keep_unused=True,
    )

    def run(in_maps):
        concat_in = [
            np.concatenate([np.asarray(m[name]) for m in in_maps], axis=0)
            for name in in_names
        ]
        concat_zeros = [
            np.zeros((n_cores * z.shape[0], *z.shape[1:]), z.dtype)
            for z in zero_outs
        ]
        out_arrs = [np.asarray(o) for o in sharded(*concat_in, *concat_zeros)]
        return [
            {name: out_arrs[i].reshape(n_cores, *out_avals[i].shape)[c]
             for i, name in enumerate(out_names)}
            for c in range(n_cores)
        ]

    return run


def kernel(x, W_qkv, b_qkv, W_o, b_o):
    """Full-input entry point: shards batch across the 8 NeuronCores,
    runs the Bass MHA kernel SPMD, gathers the full output."""
    x = np.ascontiguousarray(np.asarray(x, np.float32))
    W_qkv = np.asarray(W_qkv, np.float32)
    b_qkv = np.asarray(b_qkv, np.float32)
    W_o = np.asarray(W_o, np.float32)
    b_o = np.asarray(b_o, np.float32)
    B = x.shape[0]
    assert x.shape == (8, T, D), f"unexpected x shape {x.shape}"

    if "run" not in _CACHE:
        nc = build_nc()
        _CACHE["run"] = _make_runner(nc, n_cores=8)
    run = _CACHE["run"]

    in_maps = prep_in_maps(x, W_qkv, b_qkv, W_o, b_o)
    res = run(in_maps)
    out = np.stack([res[b]["y"] for b in range(B)]).astype(np.float32)
    return out

